# revision 1
# baseline (speedup 1.0000x reference)
"""GCN (PyG GCNConv) forward on 8 Trainium2 NeuronCores.

Reference computes z = D^-1/2 (A+I) D^-1/2 (X @ W2) + b2  (conv1 is dead code,
its result is never used).

Strategy (1D destination partition, standard distributed GCN):
  * Host: compute degrees + symmetric normalization, fold isd[src] into X,
    partition messages (edges + self loops) by destination shard
    (8 cores x 6250 nodes), build degree-sorted padded-CSR slot grids and
    int16 gather indices. The Y table is split in two halves so table-local
    indices fit in int16 (dma_gather limit: table < 32768 rows).
  * Device (identical program on all 8 cores, per-core data via in_maps):
      phase 1: GEMM  Y = XT.T @ W2  (XT pre-scaled by isd[src]) written to
               two DRAM half tables, [128, 197, 64] each.
      phase 2: pass A: dma_gather message rows from half A (messages sorted
               by dst, padded to per-tile max degree), DVE pairwise-tree
               segmented sum, partial written to DRAM. Gathers of pass A
               overlap with the half-B GEMM.
      phase 3: pass B: same gathers from half B, plus a crossmap gather of
               the pass-A partial rows (the two passes use different
               degree-sorted dst orders), combine, scale by isd[dst], out.
  * Host: inverse-permute per-core outputs into the global row order, + b2.
"""

import numpy as np

import concourse.bacc as bacc
import concourse.bass as bass
import concourse.mybir as mybir
from concourse.bass_utils import run_bass_kernel_spmd
from concourse.library_config import mlp

# ---------------- problem constants (hardcoded per contract) ----------------
N = 50000          # nodes
FIN = 128          # input channels
FOUT = 64          # output channels
NCORES = 8
PER = N // NCORES  # 6250 dst nodes per core
TILES = 49         # ceil(PER/128)
PADN = TILES * 128  # 6272 padded dst slots per core
HC = 197           # Y-table chunks (of 128 rows) per half
HALFC = HC * 128   # 25216 table rows per half
NCOL = 2 * HALFC   # 50432 XT columns (with zero pads)
ZSLOT = 40 * HC + 195  # table idx of a guaranteed-zero row (col 25000/50216)
CAP = 6144         # max num_idxs per reduce batch (48 chunk-layers)
CH_T = 8           # GEMM tiles per chunk (1024 XT cols per DMA)

_cache = {}


# ------------------------------ host schedule -------------------------------
def _node_table(n):
    """node id -> (half, table idx). Vectorized."""
    col = n + 216 * (n >= 25000)
    c = col // 128
    p = col % 128
    half = (c >= HC).astype(np.int64)
    idx = p * HC + (c - HC * half)
    return half, idx


def _wrap_idx(flat):
    """[L] int16 slot list -> [128, L//16] wrapped+replicated index array."""
    L = flat.shape[0]
    assert L % 16 == 0
    a16 = flat.reshape(L // 16, 16).T  # idx i at [i%16, i//16]
    return np.ascontiguousarray(np.tile(a16, (8, 1)))


def _build_schedule(src, dst):
    """Returns (isd, shared batch structure, per-core index arrays, outmaps)."""
    msrc = np.concatenate([src, np.arange(N, dtype=np.int64)])
    mdst = np.concatenate([dst, np.arange(N, dtype=np.int64)])
    deg = np.bincount(mdst, minlength=N)
    isd = (1.0 / np.sqrt(np.maximum(deg, 1))).astype(np.float32)

    mhalf, mtab = _node_table(msrc)
    core = mdst // PER
    dloc = mdst - core * PER

    percore = []
    Dmax = np.zeros((2, TILES), dtype=np.int64)
    for k in range(NCORES):
        entry = {}
        for h in (0, 1):
            sel = (core == k) & (mhalf == h)
            d = dloc[sel]
            t = mtab[sel]
            cnt = np.bincount(d, minlength=PER)
            order = np.argsort(cnt, kind="stable")  # ascending degree
            pos = np.empty(PER, dtype=np.int64)
            pos[order] = np.arange(PER) + (PADN - PER)  # dummies at 0..21
            o2 = np.argsort(d, kind="stable")
            ds = d[o2]
            starts = np.searchsorted(ds, np.arange(PER))
            j = np.arange(ds.shape[0]) - starts[ds]  # rank within dst
            mpos = pos[ds]
            cntpad = np.zeros(PADN, dtype=np.int64)
            cntpad[pos] = cnt
            Dmax[h] = np.maximum(Dmax[h], cntpad.reshape(TILES, 128).max(axis=1))
            entry[h] = dict(pos=pos, tile=mpos // 128, pslot=mpos % 128,
                            j=j, tab=t[o2])
        percore.append(entry)

    # shared batches per pass: consecutive tiles padded to the batch max
    # degree; extend while the padding this adds stays small and the
    # reduce-buffer cap is respected
    batches = {0: [], 1: []}
    for h in (0, 1):
        t0 = 0
        while t0 < TILES:
            g = 1
            dbatch = max(1, int(Dmax[h][t0]))
            while t0 + g < TILES:
                nd = max(dbatch, int(Dmax[h][t0 + g]))
                if nd * (g + 1) > CAP // 128:
                    break
                waste = nd * (g + 1) - (
                    dbatch * g + max(1, int(Dmax[h][t0 + g])))
                if waste > max(2, (nd * (g + 1)) // 16):
                    break
                dbatch = nd
                g += 1
            batches[h].append((t0, g, dbatch))
            t0 += g

    inmaps = []
    outmaps = []
    for k in range(NCORES):
        m = {}
        for h in (0, 1):
            e = percore[k][h]
            grids = []
            for (t0, g, db) in batches[h]:
                grid = np.full(128 * db * g, ZSLOT, dtype=np.int16)
                sel = (e["tile"] >= t0) & (e["tile"] < t0 + g)
                gg = e["tile"][sel] - t0
                lin = (gg * db + e["j"][sel]) * 128 + e["pslot"][sel]
                grid[lin] = e["tab"][sel].astype(np.int16)
                grids.append(grid)
            m["idx" + "ab"[h]] = _wrap_idx(np.concatenate(grids))
        posA = percore[k][0]["pos"]
        posB = percore[k][1]["pos"]
        xm = np.arange(PADN, dtype=np.int16)  # dummies map to themselves
        xm[posB] = posA.astype(np.int16)
        m["xmap"] = _wrap_idx(xm)
        isdb = np.zeros(PADN, dtype=np.float32)
        isdb[posB] = isd[k * PER:(k + 1) * PER]
        m["isdb"] = np.ascontiguousarray(isdb.reshape(TILES, 128).T)
        inmaps.append(m)
        om = np.full(PADN, -1, dtype=np.int64)
        om[posB] = np.arange(k * PER, (k + 1) * PER)
        outmaps.append(om)

    return isd, batches, inmaps, outmaps


# ------------------------------ device program ------------------------------
NQ = 4        # SWDGE queues (parallel Q7 descriptor generation)
NBUF = 4      # gather buffer rotation depth
PIECE_CH = 8  # chunk-layers per dma_gather piece (ring carveout limit)
CAPCH = 48    # max chunk-layers (G*D) per batch -> <= 6 pieces


def _build_program(batches, reps=1, mode="full"):
    nc = bacc.Bacc("TRN2", debug=False, num_swdge_queues=NQ)
    f32 = mybir.dt.float32
    i16 = mybir.dt.int16

    colsA = sum(128 * db * g for (_, g, db) in batches[0]) // 16
    colsB = sum(128 * db * g for (_, g, db) in batches[1]) // 16
    dgmax = max(db * g for h in (0, 1) for (_, g, db) in batches[h])
    MAXP = -(-dgmax // PIECE_CH)

    xt = nc.declare_dram_parameter("xt", [FIN, NCOL], f32, isOutput=False)
    w = nc.declare_dram_parameter("w", [FIN, FOUT], f32, isOutput=False)
    idxa = nc.declare_dram_parameter("idxa", [128, colsA], i16, isOutput=False)
    idxb = nc.declare_dram_parameter("idxb", [128, colsB], i16, isOutput=False)
    xmap = nc.declare_dram_parameter("xmap", [128, PADN // 16], i16, isOutput=False)
    isdb = nc.declare_dram_parameter("isdb", [128, TILES], f32, isOutput=False)
    out = nc.declare_dram_parameter("out", [PADN, FOUT], f32, isOutput=True)

    ya = nc.dram_tensor("ya", [128, HC, FOUT], f32)
    yb = nc.dram_tensor("yb", [128, HC, FOUT], f32)
    pa = nc.dram_tensor("pa", [PADN, FOUT], f32)
    yh = {0: ya, 1: yb}

    # per-body GEMM chunk list: (half, tile offset within half, ntiles)
    chunks1 = []
    for h in (0, 1):
        c0 = 0
        while c0 < HC:
            nt = min(CH_T, HC - c0)
            chunks1.append((h, c0, nt))
            c0 += nt
    NCH = len(chunks1)
    assert NCH % 2 == 0
    # per-body gather batch list: (pass, t0, g, db, idx col offset)
    gb1 = []
    for h in (0, 1):
        off = 0
        for (t0, g, db) in batches[h]:
            assert db * g <= CAPCH
            gb1.append((h, t0, g, db, off))
            off += db * g * 8
    NBAT = len(gb1)
    NA = len(batches[0])
    NXP = -(-TILES // PIECE_CH)

    def npieces(db, g):
        return -(-(db * g) // PIECE_CH)

    # global (repeat-extended) bookkeeping
    GCH = [(r, ci) + chunks1[ci] for r in range(reps) for ci in range(NCH)]
    GBT = [(r, bi) + gb1[bi] for r in range(reps) for bi in range(NBAT)]
    ywr_cum = [0, 0]
    ywr_after = []
    for gci in range(len(GCH)):
        ywr_cum[gci % 2] += 16
        ywr_after.append(tuple(ywr_cum))

    def dve_ops(db, h):
        n = 0
        cur = db
        while cur > 1:
            n += 1
            cur = cur - cur // 2
        n += 2 if h == 1 else 1   # combine+scale (B) / move-to-res (A)
        return n

    # Global piece sequence: pieces strictly alternate SWDGE queues so the
    # two descriptor rings ping-pong (a queue's next piece must wait for its
    # previous piece's full drain). Completion sems rotate per queue with
    # depth KK so each sem has at most one outstanding DMA.
    KK = 16
    vch_cum = 0
    vch_after = []
    pc_body = [0]  # body-local piece counter (resets per repeat)

    def alloc_piece(plist):
        gp = pc_body[0]
        pc_body[0] += 1
        q = gp % NQ
        kk = (gp // NQ) % KK
        plist.append((q, kk))
        return q, kk

    batch_pieces1 = []   # per body-local batch: [(q, kk), ...]
    xmap_pieces1 = []    # crossmap pieces
    for bi, (h, t0, g, db, off) in enumerate(gb1):
        pl = []
        for _ in range(npieces(db, g)):
            alloc_piece(pl)
        batch_pieces1.append(pl)
        if bi == NA - 1:
            for _ in range(NXP):
                alloc_piece(xmap_pieces1)
    # sem values: cumulative uses across the repeat-extended sequence
    use_cum = {}
    batch_vals = []      # per global batch: [(q, kk, val), ...]
    xmap_vals = []       # per repeat: [(q, kk, val), ...]
    for r in range(reps):
        for bi in range(NBAT):
            vals = []
            for (q, kk) in batch_pieces1[bi]:
                use_cum[(q, kk)] = use_cum.get((q, kk), 0) + 16
                vals.append((q, kk, use_cum[(q, kk)]))
            batch_vals.append(vals)
            if bi == NA - 1:
                xv = []
                for (q, kk) in xmap_pieces1:
                    use_cum[(q, kk)] = use_cum.get((q, kk), 0) + 16
                    xv.append((q, kk, use_cum[(q, kk)]))
                xmap_vals.append(xv)
    rep_end_vals = []    # all-sem snapshot at end of each repeat
    cum2 = {}
    for r in range(reps):
        for bi in range(NBAT):
            for (q, kk, val) in batch_vals[r * NBAT + bi]:
                cum2[(q, kk)] = val
            if bi == NA - 1:
                for (q, kk, val) in xmap_vals[r]:
                    cum2[(q, kk)] = val
        rep_end_vals.append(dict(cum2))
    for gbi, (r, bi, h, t0, g, db, off) in enumerate(GBT):
        vch_cum += dve_ops(db, h)
        vch_after.append(vch_cum)

    from contextlib import ExitStack
    with ExitStack() as ctx:
        w_sb = ctx.enter_context(nc.sbuf_tensor("w_sb", [FIN, FOUT], f32))
        xt_sb = ctx.enter_context(
            nc.sbuf_tensor("xt_sb", [FIN, 2, CH_T * 128], f32))
        y_sb = ctx.enter_context(
            nc.sbuf_tensor("y_sb", [128, 2, CH_T, FOUT], f32))
        ia_sb = ctx.enter_context(nc.sbuf_tensor("ia_sb", [128, colsA], i16))
        ib_sb = ctx.enter_context(nc.sbuf_tensor("ib_sb", [128, colsB], i16))
        xm_sb = ctx.enter_context(
            nc.sbuf_tensor("xm_sb", [128, PADN // 16], i16))
        isdb_sb = ctx.enter_context(
            nc.sbuf_tensor("isdb_sb", [128, TILES], f32))
        g_sb = ctx.enter_context(
            nc.sbuf_tensor("g_sb", [128, NBUF, dgmax, FOUT], f32))
        x_sb = ctx.enter_context(
            nc.sbuf_tensor("x_sb", [128, TILES, FOUT], f32))
        res_sb = ctx.enter_context(
            nc.sbuf_tensor("res_sb", [128, TILES, FOUT], f32))
        lib_scr = ctx.enter_context(nc.sbuf_tensor("lib_scr", [128, 4], i16))
        ps = ctx.enter_context(
            nc.psum_tensor("ps", [128, 2 * CH_T * FOUT], f32))
        names = (["LIB", "PRM", "XTL0", "XTL1", "MMC", "YCP", "YWR0", "YWR1",
                  "VCH", "BWPA", "BWOUT"]
                 + ["GS%d_%d" % (q, kk) for q in range(NQ) for kk in range(KK)])
        sem = {n: ctx.enter_context(nc.semaphore(n)) for n in names}
        LIB, PRM, MMC, YCP, VCH = (
            sem["LIB"], sem["PRM"], sem["MMC"], sem["YCP"], sem["VCH"])
        BWPA, BWOUT = sem["BWPA"], sem["BWOUT"]
        XTL = [sem["XTL0"], sem["XTL1"]]
        YWR = [sem["YWR0"], sem["YWR1"]]
        GS = [[sem["GS%d_%d" % (q, kk)] for kk in range(KK)]
              for q in range(NQ)]
        block = ctx.enter_context(nc.Block())

        def ywr_gate(r, h):
            gci = r * NCH + (NCH // 2 - 1 if h == 0 else NCH - 1)
            return ywr_after[gci]

        @block.gpsimd
        def _(g: bass.BassGpSimd):
            g.load_library(mlp)
            # dummy SWDGE DMA: its completion implies the ucode reload is
            # fully done (HWDGE transfers during the reload crash the device)
            g.dma_start(lib_scr[:], xmap[:, :4]).then_inc(LIB, 16)
            if mode == "gemm":
                return
            for gbi, (r, bi, h, t0, gt, db, off) in enumerate(GBT):
                if gbi == 0:
                    g.wait_ge(PRM, 16 * 5)
                if mode == "full":
                    gate = ywr_gate(r, h)
                    g.wait_ge(YWR[0], gate[0])
                    g.wait_ge(YWR[1], gate[1])
                if gbi >= NBUF and mode != "gatheronly":
                    g.wait_ge(VCH, vch_after[gbi - NBUF])  # buf free
                isb = ia_sb if h == 0 else ib_sb
                nch = db * gt
                for j, j0 in enumerate(range(0, nch, PIECE_CH)):
                    nj = min(PIECE_CH, nch - j0)
                    q, kk, _ = batch_vals[gbi][j]
                    g.dma_gather(
                        g_sb[:, gbi % NBUF, j0: j0 + nj, :],
                        yh[h][:].rearrange("p c f -> (p c) f"),
                        isb[:, off + j0 * 8: off + j0 * 8 + nj * 8],
                        128 * nj, 128 * nj, FOUT,
                        queue_num=q,
                    ).then_inc(GS[q][kk], 16)
                if bi == NA - 1:
                    # crossmap gather of this repeat's pass-A partial rows
                    if mode != "gatheronly":
                        g.wait_ge(BWPA, 16 * (r + 1))
                    for j, j0 in enumerate(range(0, TILES, PIECE_CH)):
                        nj = min(PIECE_CH, TILES - j0)
                        q, kk, _ = xmap_vals[r][j]
                        g.dma_gather(
                            x_sb[:, j0: j0 + nj, :], pa[:],
                            xm_sb[:, j0 * 8: j0 * 8 + nj * 8],
                            128 * nj, 128 * nj, FOUT,
                            queue_num=q,
                        ).then_inc(GS[q][kk], 16)

        def _emit_gemm_dmas(s, r):
            for ci in range(NCH):
                gci = r * NCH + ci
                h, c0, nt = chunks1[ci]
                if gci >= 2:
                    s.wait_ge(MMC, gci - 1)  # xt buf parity free
                col0 = (h * HC + c0) * 128
                s.dma_start(
                    xt_sb[:, gci % 2, : nt * 128],
                    xt[:, col0: col0 + nt * 128],
                ).then_inc(XTL[gci % 2], 16)
                if ci >= 1:
                    pg = gci - 1
                    ph, pc0, pnt = chunks1[ci - 1]
                    s.wait_ge(YCP, pg + 1)
                    s.dma_start(
                        yh[ph][:, pc0: pc0 + pnt, :],
                        y_sb[:, pg % 2, :pnt, :],
                    ).then_inc(YWR[pg % 2], 16)
            gci = r * NCH + NCH - 1
            lh, lc0, lnt = chunks1[NCH - 1]
            s.wait_ge(YCP, gci + 1)
            s.dma_start(
                yh[lh][:, lc0: lc0 + lnt, :], y_sb[:, gci % 2, :lnt, :]
            ).then_inc(YWR[gci % 2], 16)

        @block.sync
        def _(s: bass.BassEngine):
            s.wait_ge(LIB, 16)
            s.dma_start(w_sb[:], w[:]).then_inc(PRM, 16)
            s.dma_start(ia_sb[:], idxa[:]).then_inc(PRM, 16)
            s.dma_start(ib_sb[:], idxb[:]).then_inc(PRM, 16)
            s.dma_start(xm_sb[:], xmap[:]).then_inc(PRM, 16)
            s.dma_start(isdb_sb[:], isdb[:]).then_inc(PRM, 16)
            for r in range(reps):
                if r > 0 and mode == "full":
                    # repeat r's GEMM overwrites tables repeat r-1 reads:
                    # gate on all of r-1's gathers
                    for (q, kk), val in rep_end_vals[r - 1].items():
                        s.wait_ge(GS[q][kk], val)
                if mode not in ("gather", "gatheronly"):
                    _emit_gemm_dmas(s, r)
                if mode in ("gemm", "gatheronly"):
                    continue
                # one bulk result write per pass
                s.wait_ge(VCH, vch_after[r * NBAT + NA - 1])
                s.dma_start(
                    pa[:].rearrange("(t p) f -> p t f", p=128), res_sb[:]
                ).then_inc(BWPA, 16)
                s.wait_ge(VCH, vch_after[r * NBAT + NBAT - 1])
                s.dma_start(
                    out[:].rearrange("(t p) f -> p t f", p=128), res_sb[:]
                ).then_inc(BWOUT, 16)

        @block.tensor
        def _(t: bass.BassTensorEngine):
            if mode in ("gather", "gatheronly"):
                return
            t.wait_ge(PRM, 16 * 5)  # all param DMAs done (incl. W)
            for gci, (r, ci, h, c0, nt) in enumerate(GCH):
                t.wait_ge(XTL[gci % 2], 16 * (gci // 2 + 1))
                if gci >= 2:
                    t.wait_ge(YCP, gci - 1)  # psum bank parity free
                ins = None
                for ti in range(nt):
                    ins = t.matmul(
                        out=ps[:, (gci % 2) * CH_T * FOUT + ti * FOUT:
                               (gci % 2) * CH_T * FOUT + (ti + 1) * FOUT],
                        lhsT=xt_sb[:, gci % 2, ti * 128: (ti + 1) * 128],
                        rhs=w_sb[:],
                        start=True, stop=True,
                    )
                ins.then_inc(MMC, 1)

        @block.scalar
        def _(a: bass.BassScalarEngine):
            if mode in ("gather", "gatheronly"):
                return
            for gci, (r, ci, h, c0, nt) in enumerate(GCH):
                a.wait_ge(MMC, gci + 1)
                if gci >= 2:
                    a.wait_ge(YWR[gci % 2], ywr_after[gci - 2][gci % 2])
                base = (gci % 2) * CH_T * FOUT
                a.activation(
                    out=y_sb[:, gci % 2, :nt, :].rearrange("p c f -> p (c f)"),
                    in_=ps[:, base: base + nt * FOUT],
                    func=mybir.ActivationFunctionType.Copy,
                ).then_inc(YCP, 1)

        @block.vector
        def _(v: bass.BassVectorEngine):
            if mode in ("gemm", "gatheronly"):
                return
            vc = [0]

            def chained(fn):
                # serialize dependent same-engine DVE ops via a chain sem
                # (the DVE pipeline gives no same-engine RAW guarantee)
                if vc[0]:
                    v.wait_ge(VCH, vc[0])
                ins = fn()
                ins.then_inc(VCH, 1)
                vc[0] += 1
                return ins

            v.wait_ge(PRM, 16 * 5)
            for gbi, (r, bi, h, t0, gt, db, off) in enumerate(GBT):
                for (q, kk, val) in batch_vals[gbi]:
                    v.wait_ge(GS[q][kk], val)
                if bi == 0 and r > 0:
                    v.wait_ge(BWOUT, 16 * r)   # res_sb drained (prev repeat)
                if bi == NA:
                    v.wait_ge(BWPA, 16 * (r + 1))  # res_sb drained (pass A)
                    for (q, kk, val) in xmap_vals[r]:
                        v.wait_ge(GS[q][kk], val)
                buf = g_sb[:, gbi % NBUF, : db * gt, :].rearrange(
                    "p (g d) f -> p g d f", g=gt
                )
                res = res_sb[:, t0: t0 + gt, :]
                last = None
                cur = db
                while cur > 1:
                    half = cur // 2
                    hi = cur - half
                    last = chained(lambda half=half, hi=hi: v.tensor_tensor(
                        out=buf[:, :, 0:half, :],
                        in0=buf[:, :, 0:half, :],
                        in1=buf[:, :, hi: hi + half, :],
                        op=mybir.AluOpType.add,
                    ))
                    cur = hi
                if h == 1:
                    chained(lambda: v.tensor_tensor(
                        out=buf[:, :, 0, :],
                        in0=buf[:, :, 0, :],
                        in1=x_sb[:, t0: t0 + gt, :],
                        op=mybir.AluOpType.add,
                    ))
                    last = chained(lambda: v.tensor_tensor(
                        out=res,
                        in0=buf[:, :, 0, :],
                        in1=isdb_sb[:, t0: t0 + gt, None].to_broadcast(
                            [128, gt, FOUT]
                        ),
                        op=mybir.AluOpType.mult,
                    ))
                else:
                    # move the pass-A sum into the result buffer
                    last = chained(lambda: v.tensor_scalar_mul(
                        out=res, in0=buf[:, :, 0, :], scalar1=1.0
                    ))
                assert vc[0] == vch_after[gbi], (gbi, vc[0], vch_after[gbi])

    nc.compile()
    return nc


# --------------------------------- kernel -----------------------------------
def prepare(edges, features, W2, b2):
    """Build (nc, in_maps, assemble) for the given full inputs."""
    edges = np.asarray(edges)
    X = np.asarray(features, dtype=np.float32)
    W2 = np.asarray(W2, dtype=np.float32)
    b2 = np.asarray(b2, dtype=np.float32)
    src = edges[0].astype(np.int64)
    dst = edges[1].astype(np.int64)

    isd, batches, inmaps, outmaps = _build_schedule(src, dst)

    key = tuple((h, tuple(batches[h])) for h in (0, 1))
    if key not in _cache:
        _cache[key] = _build_program(batches)
    nc = _cache[key]

    # XT: [128, NCOL], column col(n) = isd[n] * X[n]; pad columns zero
    Xs = X * isd[:, None]
    XT = np.zeros((FIN, NCOL), dtype=np.float32)
    cols = np.arange(N) + 216 * (np.arange(N) >= 25000)
    XT[:, cols] = Xs.T

    in_maps = []
    for k in range(NCORES):
        m = dict(inmaps[k])
        m["xt"] = XT
        m["w"] = W2
        in_maps.append(m)

    def assemble(results):
        z = np.empty((N, FOUT), dtype=np.float32)
        for k in range(NCORES):
            om = outmaps[k]
            valid = om >= 0
            z[om[valid]] = results[k]["out"][valid]
        return z + b2[None, :]

    return nc, in_maps, assemble


def kernel(edges, features, W1, b1, W2, b2):
    nc, in_maps, assemble = prepare(edges, features, W2, b2)
    res = run_bass_kernel_spmd(nc, in_maps, list(range(NCORES)))
    return assemble(res.results)



# revision 5
# speedup vs baseline: 2.8961x; 2.8961x over previous
"""GCN (PyG GCNConv) forward on 8 Trainium2 NeuronCores.

Reference computes z = D^-1/2 (A+I) D^-1/2 (X @ W2) + b2  (conv1 is dead code,
its result is never used).

Strategy (1D destination partition, standard distributed GCN):
  * Host: compute degrees + symmetric normalization, fold isd[src] into X,
    partition messages (edges + self loops) by destination shard
    (8 cores x 6250 nodes), build degree-sorted padded-CSR slot grids and
    int16 gather indices. The Y table is split in two halves so table-local
    indices fit in int16 (dma_gather limit: table < 32768 rows).
  * Device (identical program on all 8 cores, per-core data via in_maps):
      phase 1: GEMM  Y = XT.T @ W2  (XT pre-scaled by isd[src]) written to
               two DRAM half tables, [128, 197, 64] each.
      phase 2: pass A: dma_gather message rows from half A (messages sorted
               by dst, padded to per-tile max degree), DVE pairwise-tree
               segmented sum, partial written to DRAM. Gathers of pass A
               overlap with the half-B GEMM.
      phase 3: pass B: same gathers from half B, plus a crossmap gather of
               the pass-A partial rows (the two passes use different
               degree-sorted dst orders), combine, scale by isd[dst], out.
  * Host: inverse-permute per-core outputs into the global row order, + b2.
"""

import numpy as np

import concourse.bacc as bacc
import concourse.bass as bass
import concourse.mybir as mybir
from concourse.bass_utils import run_bass_kernel_spmd
from concourse.library_config import mlp

# ---------------- problem constants (hardcoded per contract) ----------------
N = 50000          # nodes
FIN = 128          # input channels
FOUT = 64          # output channels
NCORES = 8
PER = N // NCORES  # 6250 dst nodes per core
TILES = 49         # ceil(PER/128)
PADN = TILES * 128  # 6272 padded dst slots per core
HC = 197           # Y-table chunks (of 128 rows) per half
HALFC = HC * 128   # 25216 table rows per half
NCOL = 2 * HALFC   # 50432 XT columns (with zero pads)
ZSLOT = 40 * HC + 195  # table idx of a guaranteed-zero row (col 25000/50216)
CAP = 6144         # max num_idxs per reduce batch (48 chunk-layers)
CH_T = 8           # GEMM tiles per chunk (1024 XT cols per DMA)

_cache = {}


# ------------------------------ host schedule -------------------------------
def _node_table(n):
    """node id -> (half, table idx). Vectorized."""
    col = n + 216 * (n >= 25000)
    c = col // 128
    p = col % 128
    half = (c >= HC).astype(np.int64)
    idx = p * HC + (c - HC * half)
    return half, idx


def _wrap_idx(flat):
    """[L] int16 slot list -> [128, L//16] wrapped+replicated index array."""
    L = flat.shape[0]
    assert L % 16 == 0
    a16 = flat.reshape(L // 16, 16).T  # idx i at [i%16, i//16]
    return np.ascontiguousarray(np.tile(a16, (8, 1)))


def _build_schedule(src, dst):
    """Returns (isd, shared batch structure, per-core index arrays, outmaps)."""
    msrc = np.concatenate([src, np.arange(N, dtype=np.int64)])
    mdst = np.concatenate([dst, np.arange(N, dtype=np.int64)])
    deg = np.bincount(mdst, minlength=N)
    isd = (1.0 / np.sqrt(np.maximum(deg, 1))).astype(np.float32)

    mhalf, mtab = _node_table(msrc)
    core = mdst // PER
    dloc = mdst - core * PER

    percore = []
    Dmax = np.zeros((2, TILES), dtype=np.int64)
    for k in range(NCORES):
        entry = {}
        for h in (0, 1):
            sel = (core == k) & (mhalf == h)
            d = dloc[sel]
            t = mtab[sel]
            cnt = np.bincount(d, minlength=PER)
            order = np.argsort(cnt, kind="stable")  # ascending degree
            pos = np.empty(PER, dtype=np.int64)
            pos[order] = np.arange(PER) + (PADN - PER)  # dummies at 0..21
            o2 = np.argsort(d, kind="stable")
            ds = d[o2]
            starts = np.searchsorted(ds, np.arange(PER))
            j = np.arange(ds.shape[0]) - starts[ds]  # rank within dst
            mpos = pos[ds]
            cntpad = np.zeros(PADN, dtype=np.int64)
            cntpad[pos] = cnt
            Dmax[h] = np.maximum(Dmax[h], cntpad.reshape(TILES, 128).max(axis=1))
            entry[h] = dict(pos=pos, tile=mpos // 128, pslot=mpos % 128,
                            j=j, tab=t[o2])
        percore.append(entry)

    # shared batches per pass: consecutive tiles padded to the batch max
    # degree; extend while the padding this adds stays small and the
    # reduce-buffer cap is respected
    batches = {0: [], 1: []}
    for h in (0, 1):
        t0 = 0
        while t0 < TILES:
            g = 1
            dbatch = max(1, int(Dmax[h][t0]))
            while t0 + g < TILES:
                nd = max(dbatch, int(Dmax[h][t0 + g]))
                if nd * (g + 1) > CAP // 128:
                    break
                waste = nd * (g + 1) - (
                    dbatch * g + max(1, int(Dmax[h][t0 + g])))
                if waste > max(2, (nd * (g + 1)) // 16):
                    break
                dbatch = nd
                g += 1
            batches[h].append((t0, g, dbatch))
            t0 += g

    inmaps = []
    outmaps = []
    for k in range(NCORES):
        m = {}
        for h in (0, 1):
            e = percore[k][h]
            grids = []
            for (t0, g, db) in batches[h]:
                grid = np.full(128 * db * g, ZSLOT, dtype=np.int16)
                sel = (e["tile"] >= t0) & (e["tile"] < t0 + g)
                gg = e["tile"][sel] - t0
                lin = (gg * db + e["j"][sel]) * 128 + e["pslot"][sel]
                grid[lin] = e["tab"][sel].astype(np.int16)
                grids.append(grid)
            m["idx" + "ab"[h]] = _wrap_idx(np.concatenate(grids))
        posA = percore[k][0]["pos"]
        posB = percore[k][1]["pos"]
        xm = np.arange(PADN, dtype=np.int16)  # dummies map to themselves
        xm[posB] = posA.astype(np.int16)
        m["xmap"] = _wrap_idx(xm)
        isdb = np.zeros(PADN, dtype=np.float32)
        isdb[posB] = isd[k * PER:(k + 1) * PER]
        m["isdb"] = np.ascontiguousarray(isdb.reshape(TILES, 128).T)
        inmaps.append(m)
        om = np.full(PADN, -1, dtype=np.int64)
        om[posB] = np.arange(k * PER, (k + 1) * PER)
        outmaps.append(om)

    return isd, batches, inmaps, outmaps


# ------------------------------ device program ------------------------------
NQ = 4        # SWDGE queues (parallel Q7 descriptor generation)
NBUF = 4      # gather buffer rotation depth
PIECE_CH = 8  # chunk-layers per dma_gather piece (ring carveout limit)
CAPCH = 48    # max chunk-layers (G*D) per batch -> <= 6 pieces


def _build_program(batches, reps=1, mode="full", piece_ch=PIECE_CH, nq=NQ,
                   scratch=16384, spkt=True):
    nc = bacc.Bacc("TRN2", debug=False, num_swdge_queues=nq,
                   dynamic_dma_scratch_size=scratch)
    f32 = mybir.dt.float32
    i16 = mybir.dt.int16

    colsA = sum(128 * db * g for (_, g, db) in batches[0]) // 16
    colsB = sum(128 * db * g for (_, g, db) in batches[1]) // 16
    dgmax = max(db * g for h in (0, 1) for (_, g, db) in batches[h])
    MAXP = -(-dgmax // piece_ch)

    xt = nc.declare_dram_parameter("xt", [FIN, NCOL], f32, isOutput=False)
    w = nc.declare_dram_parameter("w", [FIN, FOUT], f32, isOutput=False)
    idxa = nc.declare_dram_parameter("idxa", [128, colsA], i16, isOutput=False)
    idxb = nc.declare_dram_parameter("idxb", [128, colsB], i16, isOutput=False)
    xmap = nc.declare_dram_parameter("xmap", [128, PADN // 16], i16, isOutput=False)
    isdb = nc.declare_dram_parameter("isdb", [128, TILES], f32, isOutput=False)
    out = nc.declare_dram_parameter("out", [PADN, FOUT], f32, isOutput=True)

    ya = nc.dram_tensor("ya", [128, HC, FOUT], f32)
    yb = nc.dram_tensor("yb", [128, HC, FOUT], f32)
    pa = nc.dram_tensor("pa", [PADN, FOUT], f32)
    yh = {0: ya, 1: yb}

    # per-body GEMM chunk list: (half, tile offset within half, ntiles)
    chunks1 = []
    for h in (0, 1):
        c0 = 0
        while c0 < HC:
            nt = min(CH_T, HC - c0)
            chunks1.append((h, c0, nt))
            c0 += nt
    NCH = len(chunks1)
    assert NCH % 2 == 0
    # per-body gather batch list: (pass, t0, g, db, idx col offset)
    gb1 = []
    for h in (0, 1):
        off = 0
        for (t0, g, db) in batches[h]:
            assert db * g <= CAPCH
            gb1.append((h, t0, g, db, off))
            off += db * g * 8
    NBAT = len(gb1)
    NA = len(batches[0])
    NXP = -(-TILES // piece_ch)

    def npieces(db, g):
        return -(-(db * g) // piece_ch)

    # global (repeat-extended) bookkeeping
    GCH = [(r, ci) + chunks1[ci] for r in range(reps) for ci in range(NCH)]
    GBT = [(r, bi) + gb1[bi] for r in range(reps) for bi in range(NBAT)]
    ywr_cum = [0, 0]
    ywr_after = []
    for gci in range(len(GCH)):
        ywr_cum[gci % 2] += 16
        ywr_after.append(tuple(ywr_cum))

    def dve_ops(db, h):
        n = 0
        cur = db
        while cur > 1:
            n += 1
            cur = cur - cur // 2
        n += 2 if h == 1 else 1   # combine+scale (B) / move-to-res (A)
        return n

    # Global piece sequence: pieces strictly alternate SWDGE queues so the
    # two descriptor rings ping-pong (a queue's next piece must wait for its
    # previous piece's full drain). Completion sems rotate per queue with
    # depth KK so each sem has at most one outstanding DMA.
    KK = 16
    vch_cum = 0
    vch_after = []
    pc_body = [0]  # body-local piece counter (resets per repeat)

    def alloc_piece(plist):
        gp = pc_body[0]
        pc_body[0] += 1
        q = gp % nq
        kk = (gp // nq) % KK
        plist.append((q, kk))
        return q, kk

    batch_pieces1 = []   # per body-local batch: [(q, kk), ...]
    xmap_pieces1 = []    # crossmap pieces
    for bi, (h, t0, g, db, off) in enumerate(gb1):
        pl = []
        for _ in range(npieces(db, g)):
            alloc_piece(pl)
        batch_pieces1.append(pl)
        if bi == NA - 1:
            for _ in range(NXP):
                alloc_piece(xmap_pieces1)
    # sem values: cumulative uses across the repeat-extended sequence
    use_cum = {}
    batch_vals = []      # per global batch: [(q, kk, val), ...]
    xmap_vals = []       # per repeat: [(q, kk, val), ...]
    for r in range(reps):
        for bi in range(NBAT):
            vals = []
            for (q, kk) in batch_pieces1[bi]:
                use_cum[(q, kk)] = use_cum.get((q, kk), 0) + 16
                vals.append((q, kk, use_cum[(q, kk)]))
            batch_vals.append(vals)
            if bi == NA - 1:
                xv = []
                for (q, kk) in xmap_pieces1:
                    use_cum[(q, kk)] = use_cum.get((q, kk), 0) + 16
                    xv.append((q, kk, use_cum[(q, kk)]))
                xmap_vals.append(xv)
    rep_end_vals = []    # all-sem snapshot at end of each repeat
    cum2 = {}
    for r in range(reps):
        for bi in range(NBAT):
            for (q, kk, val) in batch_vals[r * NBAT + bi]:
                cum2[(q, kk)] = val
            if bi == NA - 1:
                for (q, kk, val) in xmap_vals[r]:
                    cum2[(q, kk)] = val
        rep_end_vals.append(dict(cum2))
    for gbi, (r, bi, h, t0, g, db, off) in enumerate(GBT):
        vch_cum += dve_ops(db, h)
        vch_after.append(vch_cum)

    from contextlib import ExitStack
    with ExitStack() as ctx:
        w_sb = ctx.enter_context(nc.sbuf_tensor("w_sb", [FIN, FOUT], f32))
        xt_sb = ctx.enter_context(
            nc.sbuf_tensor("xt_sb", [FIN, 2, CH_T * 128], f32))
        y_sb = ctx.enter_context(
            nc.sbuf_tensor("y_sb", [128, 2, CH_T, FOUT], f32))
        ia_sb = ctx.enter_context(nc.sbuf_tensor("ia_sb", [128, colsA], i16))
        ib_sb = ctx.enter_context(nc.sbuf_tensor("ib_sb", [128, colsB], i16))
        xm_sb = ctx.enter_context(
            nc.sbuf_tensor("xm_sb", [128, PADN // 16], i16))
        isdb_sb = ctx.enter_context(
            nc.sbuf_tensor("isdb_sb", [128, TILES], f32))
        g_sb = ctx.enter_context(
            nc.sbuf_tensor("g_sb", [128, NBUF, dgmax, FOUT], f32))
        x_sb = ctx.enter_context(
            nc.sbuf_tensor("x_sb", [128, TILES, FOUT], f32))
        res_sb = ctx.enter_context(
            nc.sbuf_tensor("res_sb", [128, TILES, FOUT], f32))
        lib_scr = ctx.enter_context(nc.sbuf_tensor("lib_scr", [128, 4], i16))
        ps = ctx.enter_context(
            nc.psum_tensor("ps", [128, 2 * CH_T * FOUT], f32))
        names = (["LIB", "PRM", "XTL0", "XTL1", "MMC", "YCP", "YWR0", "YWR1",
                  "VCH", "BWPA", "BWOUT"]
                 + ["GS%d_%d" % (q, kk) for q in range(nq) for kk in range(KK)])
        sem = {n: ctx.enter_context(nc.semaphore(n)) for n in names}
        LIB, PRM, MMC, YCP, VCH = (
            sem["LIB"], sem["PRM"], sem["MMC"], sem["YCP"], sem["VCH"])
        BWPA, BWOUT = sem["BWPA"], sem["BWOUT"]
        XTL = [sem["XTL0"], sem["XTL1"]]
        YWR = [sem["YWR0"], sem["YWR1"]]
        GS = [[sem["GS%d_%d" % (q, kk)] for kk in range(KK)]
              for q in range(nq)]
        block = ctx.enter_context(nc.Block())

        def ywr_gate(r, h):
            gci = r * NCH + (NCH // 2 - 1 if h == 0 else NCH - 1)
            return ywr_after[gci]

        @block.gpsimd
        def _(g: bass.BassGpSimd):
            g.load_library(mlp)
            # dummy SWDGE DMA: its completion implies the ucode reload is
            # fully done (HWDGE transfers during the reload crash the device)
            g.dma_start(lib_scr[:], xmap[:, :4]).then_inc(LIB, 16)
            if mode == "gemm":
                return
            for gbi, (r, bi, h, t0, gt, db, off) in enumerate(GBT):
                if gbi == 0:
                    g.wait_ge(PRM, 16 * 5)
                if mode == "full":
                    gate = ywr_gate(r, h)
                    g.wait_ge(YWR[0], gate[0])
                    g.wait_ge(YWR[1], gate[1])
                if gbi >= NBUF and mode != "gatheronly":
                    g.wait_ge(VCH, vch_after[gbi - NBUF])  # buf free
                isb = ia_sb if h == 0 else ib_sb
                nch = db * gt
                for j, j0 in enumerate(range(0, nch, piece_ch)):
                    nj = min(piece_ch, nch - j0)
                    q, kk, _ = batch_vals[gbi][j]
                    g.dma_gather(
                        g_sb[:, gbi % NBUF, j0: j0 + nj, :],
                        yh[h][:].rearrange("p c f -> (p c) f"),
                        isb[:, off + j0 * 8: off + j0 * 8 + nj * 8],
                        128 * nj, 128 * nj, FOUT,
                        queue_num=q, single_packet=spkt,
                    ).then_inc(GS[q][kk], 16)
                if bi == NA - 1:
                    # crossmap gather of this repeat's pass-A partial rows
                    if mode != "gatheronly":
                        g.wait_ge(BWPA, 16 * (r + 1))
                    for j, j0 in enumerate(range(0, TILES, piece_ch)):
                        nj = min(piece_ch, TILES - j0)
                        q, kk, _ = xmap_vals[r][j]
                        g.dma_gather(
                            x_sb[:, j0: j0 + nj, :], pa[:],
                            xm_sb[:, j0 * 8: j0 * 8 + nj * 8],
                            128 * nj, 128 * nj, FOUT,
                            queue_num=q, single_packet=spkt,
                        ).then_inc(GS[q][kk], 16)

        def _emit_gemm_dmas(s, r):
            for ci in range(NCH):
                gci = r * NCH + ci
                h, c0, nt = chunks1[ci]
                if gci >= 2:
                    s.wait_ge(MMC, gci - 1)  # xt buf parity free
                col0 = (h * HC + c0) * 128
                s.dma_start(
                    xt_sb[:, gci % 2, : nt * 128],
                    xt[:, col0: col0 + nt * 128],
                ).then_inc(XTL[gci % 2], 16)
                if ci >= 1:
                    pg = gci - 1
                    ph, pc0, pnt = chunks1[ci - 1]
                    s.wait_ge(YCP, pg + 1)
                    s.dma_start(
                        yh[ph][:, pc0: pc0 + pnt, :],
                        y_sb[:, pg % 2, :pnt, :],
                    ).then_inc(YWR[pg % 2], 16)
            gci = r * NCH + NCH - 1
            lh, lc0, lnt = chunks1[NCH - 1]
            s.wait_ge(YCP, gci + 1)
            s.dma_start(
                yh[lh][:, lc0: lc0 + lnt, :], y_sb[:, gci % 2, :lnt, :]
            ).then_inc(YWR[gci % 2], 16)

        @block.sync
        def _(s: bass.BassEngine):
            s.wait_ge(LIB, 16)
            s.dma_start(w_sb[:], w[:]).then_inc(PRM, 16)
            s.dma_start(ia_sb[:], idxa[:]).then_inc(PRM, 16)
            s.dma_start(ib_sb[:], idxb[:]).then_inc(PRM, 16)
            s.dma_start(xm_sb[:], xmap[:]).then_inc(PRM, 16)
            s.dma_start(isdb_sb[:], isdb[:]).then_inc(PRM, 16)
            for r in range(reps):
                if r > 0 and mode == "full":
                    # repeat r's GEMM overwrites tables repeat r-1 reads:
                    # gate on all of r-1's gathers
                    for (q, kk), val in rep_end_vals[r - 1].items():
                        s.wait_ge(GS[q][kk], val)
                if mode not in ("gather", "gatheronly"):
                    _emit_gemm_dmas(s, r)
                if mode in ("gemm", "gatheronly"):
                    continue
                # one bulk result write per pass
                s.wait_ge(VCH, vch_after[r * NBAT + NA - 1])
                s.dma_start(
                    pa[:].rearrange("(t p) f -> p t f", p=128), res_sb[:]
                ).then_inc(BWPA, 16)
                s.wait_ge(VCH, vch_after[r * NBAT + NBAT - 1])
                s.dma_start(
                    out[:].rearrange("(t p) f -> p t f", p=128), res_sb[:]
                ).then_inc(BWOUT, 16)

        @block.tensor
        def _(t: bass.BassTensorEngine):
            if mode in ("gather", "gatheronly"):
                return
            t.wait_ge(PRM, 16 * 5)  # all param DMAs done (incl. W)
            for gci, (r, ci, h, c0, nt) in enumerate(GCH):
                t.wait_ge(XTL[gci % 2], 16 * (gci // 2 + 1))
                if gci >= 2:
                    t.wait_ge(YCP, gci - 1)  # psum bank parity free
                ins = None
                for ti in range(nt):
                    ins = t.matmul(
                        out=ps[:, (gci % 2) * CH_T * FOUT + ti * FOUT:
                               (gci % 2) * CH_T * FOUT + (ti + 1) * FOUT],
                        lhsT=xt_sb[:, gci % 2, ti * 128: (ti + 1) * 128],
                        rhs=w_sb[:],
                        start=True, stop=True,
                    )
                ins.then_inc(MMC, 1)

        @block.scalar
        def _(a: bass.BassScalarEngine):
            if mode in ("gather", "gatheronly"):
                return
            for gci, (r, ci, h, c0, nt) in enumerate(GCH):
                a.wait_ge(MMC, gci + 1)
                if gci >= 2:
                    a.wait_ge(YWR[gci % 2], ywr_after[gci - 2][gci % 2])
                base = (gci % 2) * CH_T * FOUT
                a.activation(
                    out=y_sb[:, gci % 2, :nt, :].rearrange("p c f -> p (c f)"),
                    in_=ps[:, base: base + nt * FOUT],
                    func=mybir.ActivationFunctionType.Copy,
                ).then_inc(YCP, 1)

        @block.vector
        def _(v: bass.BassVectorEngine):
            if mode in ("gemm", "gatheronly"):
                return
            vc = [0]

            def chained(fn):
                # serialize dependent same-engine DVE ops via a chain sem
                # (the DVE pipeline gives no same-engine RAW guarantee)
                if vc[0]:
                    v.wait_ge(VCH, vc[0])
                ins = fn()
                ins.then_inc(VCH, 1)
                vc[0] += 1
                return ins

            v.wait_ge(PRM, 16 * 5)
            for gbi, (r, bi, h, t0, gt, db, off) in enumerate(GBT):
                for (q, kk, val) in batch_vals[gbi]:
                    v.wait_ge(GS[q][kk], val)
                if bi == 0 and r > 0:
                    v.wait_ge(BWOUT, 16 * r)   # res_sb drained (prev repeat)
                if bi == NA:
                    v.wait_ge(BWPA, 16 * (r + 1))  # res_sb drained (pass A)
                    for (q, kk, val) in xmap_vals[r]:
                        v.wait_ge(GS[q][kk], val)
                buf = g_sb[:, gbi % NBUF, : db * gt, :].rearrange(
                    "p (g d) f -> p g d f", g=gt
                )
                res = res_sb[:, t0: t0 + gt, :]
                last = None
                cur = db
                while cur > 1:
                    half = cur // 2
                    hi = cur - half
                    last = chained(lambda half=half, hi=hi: v.tensor_tensor(
                        out=buf[:, :, 0:half, :],
                        in0=buf[:, :, 0:half, :],
                        in1=buf[:, :, hi: hi + half, :],
                        op=mybir.AluOpType.add,
                    ))
                    cur = hi
                if h == 1:
                    chained(lambda: v.tensor_tensor(
                        out=buf[:, :, 0, :],
                        in0=buf[:, :, 0, :],
                        in1=x_sb[:, t0: t0 + gt, :],
                        op=mybir.AluOpType.add,
                    ))
                    last = chained(lambda: v.tensor_tensor(
                        out=res,
                        in0=buf[:, :, 0, :],
                        in1=isdb_sb[:, t0: t0 + gt, None].to_broadcast(
                            [128, gt, FOUT]
                        ),
                        op=mybir.AluOpType.mult,
                    ))
                else:
                    # move the pass-A sum into the result buffer
                    last = chained(lambda: v.tensor_scalar_mul(
                        out=res, in0=buf[:, :, 0, :], scalar1=1.0
                    ))
                assert vc[0] == vch_after[gbi], (gbi, vc[0], vch_after[gbi])

    nc.compile()
    return nc


# --------------------------------- kernel -----------------------------------
def prepare(edges, features, W2, b2):
    """Build (nc, in_maps, assemble) for the given full inputs."""
    edges = np.asarray(edges)
    X = np.asarray(features, dtype=np.float32)
    W2 = np.asarray(W2, dtype=np.float32)
    b2 = np.asarray(b2, dtype=np.float32)
    src = edges[0].astype(np.int64)
    dst = edges[1].astype(np.int64)

    isd, batches, inmaps, outmaps = _build_schedule(src, dst)

    key = tuple((h, tuple(batches[h])) for h in (0, 1))
    if key not in _cache:
        _cache[key] = _build_program(batches)
    nc = _cache[key]

    # XT: [128, NCOL], column col(n) = isd[n] * X[n]; pad columns zero
    Xs = X * isd[:, None]
    XT = np.zeros((FIN, NCOL), dtype=np.float32)
    cols = np.arange(N) + 216 * (np.arange(N) >= 25000)
    XT[:, cols] = Xs.T

    in_maps = []
    for k in range(NCORES):
        m = dict(inmaps[k])
        m["xt"] = XT
        m["w"] = W2
        in_maps.append(m)

    def assemble(results):
        z = np.empty((N, FOUT), dtype=np.float32)
        for k in range(NCORES):
            om = outmaps[k]
            valid = om >= 0
            z[om[valid]] = results[k]["out"][valid]
        return z + b2[None, :]

    return nc, in_maps, assemble


def kernel(edges, features, W1, b1, W2, b2):
    nc, in_maps, assemble = prepare(edges, features, W2, b2)
    res = run_bass_kernel_spmd(nc, in_maps, list(range(NCORES)))
    return assemble(res.results)



# revision 6
# speedup vs baseline: 2.9302x; 1.0118x over previous
"""GCN (PyG GCNConv) forward on 8 Trainium2 NeuronCores — v2.

Reference computes z = D^-1/2 (A+I) D^-1/2 (X @ W2) + b2  (conv1 is dead code).

v2 design (single-pass bf16 pair-table gather):
  * Y table rows hold TWO node rows ([y[a]|y[b]], 128 bf16 = 256 B, the
    minimum dma_gather elem size). 25216 rows < 32768 -> int16 indices fit
    in ONE table: single gather pass, no crossmap merge.
  * Nodes are 2-colored (balanced per dst neighborhood, greedy) so each
    dst's messages alternate half0/half1 by slot parity. The half-select
    then needs NO mask: fold level L0 is one contiguous bf16 tensor_tensor
    add of strided slices (even slots' low half + odd slots' high half).
  * XT, W, Y all bf16 (GEMM DMA halved, PE 2x rate); DVE tree in bf16 (2x).
  * Pad slots gather from a rotating pool of ~200 zero rows per parity
    (a single hot pad row serializes one DRAM bank and costs ~40%).
  * XT stays resident in SBUF across repeats (~98 KB/partition); per-repeat
    GEMM traffic is just the 6.5 MB bf16 table write, fully hidden under
    the gathers. Y tables double-buffered by repeat parity so repeat r+1's
    GEMM overlaps repeat r's gathers.
  * Gathers: 4 SWDGE queues (the ucode drains ~one SDMA engine per queue,
    ~29 GB/s each -> the 4-queue drain rate is the kernel's wall), 2048-idx
    pieces ping-ponged across queues, single_packet=False.
"""

import ml_dtypes
import numpy as np

import concourse.bacc as bacc
import concourse.bass as bass
import concourse.mybir as mybir
from concourse.bass_utils import run_bass_kernel_spmd
from concourse.library_config import mlp

# ---------------- problem constants (hardcoded per contract) ----------------
N = 50000          # nodes
FIN = 128          # input channels
FOUT = 64          # output channels
NCORES = 8
PER = N // NCORES  # 6250 dst nodes per core
TILES = 49         # ceil(PER/128)
PADN = TILES * 128  # 6272 padded dst slots per core
HC = 197           # pair-chunks: table rows = 128*HC = 25216, XT cols = 50432
NROW = 128 * HC    # 25216 pair rows
NCOL = 2 * NROW    # 50432 XT columns (two colors interleaved by 128-tiles)
ZSLOT = NROW - 1   # last row: both halves are zero-pad columns
CAP = 6144         # max num_idxs per reduce batch (48 chunk-layers)
CH_T = 8           # GEMM tiles per chunk (1024 XT cols per DMA)

NQ = 4        # SWDGE queues
NBUF = 4      # gather buffer rotation depth
PIECE_CH = 16  # chunk-layers (128 idxs each) per dma_gather piece

_cache = {}


# ------------------------------ host schedule -------------------------------
def _wrap_idx(flat):
    """[L] int16 slot list -> [128, L//16] wrapped+replicated index array."""
    L = flat.shape[0]
    assert L % 16 == 0
    a16 = flat.reshape(L // 16, 16).T  # idx i at [i%16, i//16]
    return np.ascontiguousarray(np.tile(a16, (8, 1)))


def _color_nodes(src, dst, refine=True):
    """Greedy balanced 2-coloring: each dst's in-neighborhood (incl. self
    loop) splits as evenly as possible between the two colors; global count
    per color capped at NROW. Returns color[n] in {0,1}."""
    # out-adjacency: node n -> list of dsts it messages (its out-edges + its
    # own self loop)
    osrc = np.concatenate([src, np.arange(N, dtype=np.int64)])
    odst = np.concatenate([dst, np.arange(N, dtype=np.int64)])
    order = np.argsort(osrc, kind="stable")
    osrc_s = osrc[order]
    odst_s = odst[order]
    starts = np.searchsorted(osrc_s, np.arange(N + 1))
    delta = np.zeros(N, dtype=np.int32)   # c0(d) - c1(d) so far
    color = np.zeros(N, dtype=np.int8)
    cnt = [0, 0]
    rng = np.random.default_rng(0)
    perm = rng.permutation(N)
    for n in perm:
        ds = odst_s[starts[n]:starts[n + 1]]
        t = int(np.sign(delta[ds]).sum())
        if t > 0:
            c = 1
        elif t < 0:
            c = 0
        else:
            c = 0 if cnt[0] <= cnt[1] else 1
        if cnt[c] >= NROW - 1:  # keep the last row of each color for ZSLOT
            c = 1 - c
        color[n] = c
        cnt[c] += 1
        delta[ds] += 1 - 2 * c

    # refinement: flip nodes when it reduces total slot excess
    # cost(delta) = max(delta - 1, -delta)  (excess slots for that dst)
    def cost(dl):
        return np.maximum(dl - 1, -dl)

    for _ in range(4 if refine else 0):
        flipped = 0
        for n in perm:
            ds = odst_s[starts[n]:starts[n + 1]]
            c = color[n]
            sgn = 2 * (1 - 2 * int(c))   # flip c0->c1: delta -= 2; else += 2
            dl = delta[ds]
            gain = int((cost(dl - sgn) - cost(dl)).sum())
            # keep both colors >= ~100 below capacity so the zero-row pools
            # (pad-gather spread) stay wide
            if gain < 0 and cnt[1 - c] < NROW - 116:
                color[n] = 1 - c
                cnt[c] -= 1
                cnt[1 - c] += 1
                delta[ds] = dl - sgn
                flipped += 1
        if flipped < N // 500:
            break
    return color


def _build_schedule(src, dst, hc=HC, refine=True, even_db=False):
    """Returns (isd, col0/col1 node lists, batches, per-core inmaps, outmaps).

    Slot grid: per dst, parity-0 messages at even depths, parity-1 at odd
    depths; depth(d) = max(2*c0-1, 2*c1). Single degree-sorted slot order.
    """
    msrc = np.concatenate([src, np.arange(N, dtype=np.int64)])
    mdst = np.concatenate([dst, np.arange(N, dtype=np.int64)])
    deg = np.bincount(mdst, minlength=N)
    isd = (1.0 / np.sqrt(np.maximum(deg, 1))).astype(np.float32)

    color = _color_nodes(src, dst, refine=refine)
    # node -> position within its color list (order = node id ascending)
    n0 = int((color == 0).sum())
    n1 = N - n0
    pos_in_color = np.zeros(N, dtype=np.int64)
    pos_in_color[color == 0] = np.arange(n0)
    pos_in_color[color == 1] = np.arange(n1)
    # table row of node n: row = (pos%128)*HC + pos//128 ; parity = color
    trow = (pos_in_color % 128) * hc + pos_in_color // 128
    # zero rows per parity: tail positions of each color list are zero-pad
    # columns; spread pad-slot gathers across them to avoid a hot DRAM row
    zp0 = np.arange(n0, NROW)
    zp1 = np.arange(n1, NROW)
    zrows0 = ((zp0 % 128) * hc + zp0 // 128).astype(np.int16)
    zrows1 = ((zp1 % 128) * hc + zp1 // 128).astype(np.int16)
    mrow = trow[msrc].astype(np.int16)
    mpar = color[msrc].astype(np.int64)

    core = mdst // PER
    dloc = mdst - core * PER

    # per-(core,dst) parity counts -> depth
    c0 = np.zeros((NCORES, PER), dtype=np.int64)
    c1 = np.zeros((NCORES, PER), dtype=np.int64)
    np.add.at(c0, (core[mpar == 0], dloc[mpar == 0]), 1)
    np.add.at(c1, (core[mpar == 1], dloc[mpar == 1]), 1)
    depth = np.maximum(2 * c0 - 1, 2 * c1)  # >= 1 (self loop)

    # shared slot order + batches over the max depth across cores (shared
    # program => shared batch structure; per-core Dmax merged)
    percore = []
    Dmax = np.zeros(TILES, dtype=np.int64)
    for k in range(NCORES):
        dep = depth[k]
        order = np.argsort(dep, kind="stable")
        pos = np.empty(PER, dtype=np.int64)
        pos[order] = np.arange(PER) + (PADN - PER)  # dummies at low slots
        deppad = np.zeros(PADN, dtype=np.int64)
        deppad[pos] = dep
        Dmax = np.maximum(Dmax, deppad.reshape(TILES, 128).max(axis=1))
        percore.append(dict(pos=pos))

    # batches: group consecutive tiles; db = even-rounded max depth
    batches = []
    t0 = 0
    Dv = [max(2, int(v)) for v in Dmax]
    if even_db:
        Dv = [v + (v % 2) for v in Dv]
    while t0 < TILES:
        g = 1
        dbatch = Dv[t0]
        while t0 + g < TILES:
            nd = max(dbatch, Dv[t0 + g])
            if nd * (g + 1) > CAP // 128:
                break
            waste = nd * (g + 1) - (dbatch * g + Dv[t0 + g])
            if waste > max(2, (nd * (g + 1)) // 16):
                break
            dbatch = nd
            g += 1
        batches.append((t0, g, dbatch))
        t0 += g

    inmaps = []
    outmaps = []
    for k in range(NCORES):
        pos = percore[k]["pos"]
        sel = core == k
        d = dloc[sel]
        row = mrow[sel]
        par = mpar[sel]
        o2 = np.argsort(d, kind="stable")
        ds, rows, pars = d[o2], row[o2], par[o2]
        starts = np.searchsorted(ds, np.arange(PER))
        # depth slot within dst: parity-0 messages -> 0,2,4..; parity-1 ->
        # 1,3,5..  (rank within same (dst,parity))
        key = ds * 2 + pars
        o3 = np.argsort(key, kind="stable")
        ks = key[o3]
        kstart = np.searchsorted(ks, np.arange(2 * PER))
        rank = np.arange(ds.shape[0]) - kstart[ks]
        jo = 2 * rank + (ks % 2)
        j = np.empty_like(jo)
        j[o3] = jo
        mpos = pos[ds]
        tile = mpos // 128
        pslot = mpos % 128

        grids = []
        zi = [0, 0]
        for (bt0, g, db) in batches:
            ncell = 128 * db * g
            # pads: even depth slots read zrows0 (half0 zero), odd read
            # zrows1; rotate through the zero rows
            cj = (np.arange(ncell) // 128) % db  # depth of each cell
            grid = np.empty(ncell, dtype=np.int16)
            ev = (cj % 2) == 0
            ne = int(ev.sum())
            grid[ev] = zrows0[(zi[0] + np.arange(ne)) % len(zrows0)]
            grid[~ev] = zrows1[(zi[1] + np.arange(ncell - ne)) % len(zrows1)]
            zi[0] = (zi[0] + ne) % len(zrows0)
            zi[1] = (zi[1] + ncell - ne) % len(zrows1)
            selb = (tile >= bt0) & (tile < bt0 + g)
            gg = tile[selb] - bt0
            lin = (gg * db + j[selb]) * 128 + pslot[selb]
            assert (j[selb] < db).all()
            grid[lin] = rows[selb]
            grids.append(grid)
        m = {"idx": _wrap_idx(np.concatenate(grids))}
        isdb = np.zeros(PADN, dtype=np.float32)
        isdb[pos] = isd[k * PER:(k + 1) * PER]
        m["isdb"] = np.ascontiguousarray(isdb.reshape(TILES, 128).T)
        inmaps.append(m)
        om = np.full(PADN, -1, dtype=np.int64)
        om[pos] = np.arange(k * PER, (k + 1) * PER)
        outmaps.append(om)

    return isd, color, pos_in_color, batches, inmaps, outmaps


# ------------------------------ device program ------------------------------
def _build_program(batches, reps=1, mode="full", piece_ch=PIECE_CH, nq=NQ,
                   spkt=False, nbuf=NBUF, tab32=False, ntab=None, hc=HC,
                   xtres=True):
    nc = bacc.Bacc("TRN2", debug=False, num_swdge_queues=nq)
    f32 = mybir.dt.float32
    bf16 = mybir.dt.bfloat16
    i16 = mybir.dt.int16

    cols = sum(128 * db * g for (_, g, db) in batches) // 16
    dgmax = max(db * g for (_, g, db) in batches)
    assert dgmax <= CAP // 128

    xt = nc.declare_dram_parameter("xt", [FIN, NCOL], bf16, isOutput=False)
    w = nc.declare_dram_parameter("w", [FIN, FOUT], bf16, isOutput=False)
    idx = nc.declare_dram_parameter("idx", [128, cols], i16, isOutput=False)
    isdb = nc.declare_dram_parameter("isdb", [128, TILES], f32, isOutput=False)
    out = nc.declare_dram_parameter("out", [PADN, FOUT], f32, isOutput=True)

    if ntab is None:
        ntab = 2 if reps > 1 else 1
    tdt, tew = (f32, FOUT) if tab32 else (bf16, 128)
    tabs = [nc.dram_tensor("ytab%d" % i, [128, hc, tew], tdt)
            for i in range(ntab)]

    # GEMM chunk list: chunk = CH_T tiles = CH_T/2 pair-chunks
    chunks1 = []
    c0 = 0
    while c0 < 2 * HC:  # in units of tiles (394 tiles)
        nt = min(CH_T, 2 * HC - c0)
        chunks1.append((c0, nt))
        c0 += nt
    NCH = len(chunks1)
    # gather batches with idx col offsets
    gb1 = []
    off = 0
    for (t0, g, db) in batches:
        gb1.append((t0, g, db, off))
        off += db * g * 8
    NBAT = len(gb1)

    def npieces(db, g):
        return -(-(db * g) // piece_ch)

    GCH = [(r, ci) + chunks1[ci] for r in range(reps) for ci in range(NCH)]
    GBT = [(r, bi) + gb1[bi] for r in range(reps) for bi in range(NBAT)]

    def dve_ops(db):
        n = 1  # L0 parity fold (pairs of slots)
        if db % 2:
            n += 1  # odd tail slot (low half) folded in
        cur = db // 2
        while cur > 1:
            n += 1
            cur = cur - cur // 2
        n += 1  # isd scale
        return n

    vch_after = []
    vc = 0
    for (r, bi, t0, g, db, off) in GBT:
        vc += dve_ops(db)
        vch_after.append(vc)
    VTOT = vch_after[NBAT - 1]  # per rep

    # piece -> (queue, sem slot) with strict queue alternation
    KK = 16
    pc_body = [0]

    def alloc_piece(plist):
        gp = pc_body[0]
        pc_body[0] += 1
        q = gp % nq
        kk = (gp // nq) % KK
        plist.append((q, kk))
        return q, kk

    batch_pieces1 = []
    for bi, (t0, g, db, off) in enumerate(gb1):
        pl = []
        for _ in range(npieces(db, g)):
            alloc_piece(pl)
        batch_pieces1.append(pl)
    use_cum = {}
    batch_vals = []
    for r in range(reps):
        for bi in range(NBAT):
            vals = []
            for (q, kk) in batch_pieces1[bi]:
                use_cum[(q, kk)] = use_cum.get((q, kk), 0) + 16
                vals.append((q, kk, use_cum[(q, kk)]))
            batch_vals.append(vals)
    rep_end_vals = []
    cum2 = {}
    for r in range(reps):
        for bi in range(NBAT):
            for (q, kk, val) in batch_vals[r * NBAT + bi]:
                cum2[(q, kk)] = val
        rep_end_vals.append(dict(cum2))

    from contextlib import ExitStack
    with ExitStack() as ctx:
        w_sb = ctx.enter_context(nc.sbuf_tensor("w_sb", [FIN, FOUT], bf16))
        if xtres:
            xt_sb = ctx.enter_context(
                nc.sbuf_tensor("xt_sb", [FIN, NCOL], bf16))
        else:
            xt_sb = ctx.enter_context(
                nc.sbuf_tensor("xt_sb", [FIN, 2, CH_T * 128], bf16))
        y_sb = ctx.enter_context(
            nc.sbuf_tensor("y_sb", [128, 2, CH_T * FOUT], bf16))
        idx_sb = ctx.enter_context(nc.sbuf_tensor("idx_sb", [128, cols], i16))
        isdb_sb = ctx.enter_context(
            nc.sbuf_tensor("isdb_sb", [128, TILES], f32))
        # per rotation buffer: gather slots (dgmax*128) + fold area
        # (dgmax/2*64)
        FOLD0 = dgmax * 128
        BUFW = FOLD0 + (dgmax // 2) * FOUT
        g_sb = ctx.enter_context(
            nc.sbuf_tensor("g_sb", [128, nbuf, BUFW], bf16))
        res_sb = ctx.enter_context(
            nc.sbuf_tensor("res_sb", [128, 2, TILES, FOUT], f32))
        lib_scr = ctx.enter_context(nc.sbuf_tensor("lib_scr", [128, 4], i16))
        ps = ctx.enter_context(
            nc.psum_tensor("ps", [128, 2 * CH_T * FOUT], f32))
        names = (["LIB", "PRM", "XTL0", "XTL1", "MMC", "YCP", "YWR",
                  "VCH", "BWOUT"]
                 + ["GS%d_%d" % (q, kk) for q in range(nq) for kk in range(KK)])
        sem = {n: ctx.enter_context(nc.semaphore(n)) for n in names}
        LIB, PRM, MMC, YCP, VCH = (
            sem["LIB"], sem["PRM"], sem["MMC"], sem["YCP"], sem["VCH"])
        YWR, BWOUT = sem["YWR"], sem["BWOUT"]
        XTL = [sem["XTL0"], sem["XTL1"]]
        GS = [[sem["GS%d_%d" % (q, kk)] for kk in range(KK)]
              for q in range(nq)]
        block = ctx.enter_context(nc.Block())

        @block.gpsimd
        def _(g: bass.BassGpSimd):
            g.load_library(mlp)
            g.dma_start(lib_scr[:], idx[:, :4]).then_inc(LIB, 16)
            if mode == "gemm":
                return
            for gbi, (r, bi, t0, gt, db, off) in enumerate(GBT):
                if gbi == 0:
                    g.wait_ge(PRM, 16 * 3)
                if mode == "full":
                    g.wait_ge(YWR, 16 * NCH * (r + 1))
                if gbi >= nbuf and mode != "gatheronly":
                    g.wait_ge(VCH, vch_after[(gbi - nbuf) % NBAT]
                              + ((gbi - nbuf) // NBAT) * VTOT)
                nch = db * gt
                tab = tabs[r % ntab]
                for jp, j0 in enumerate(range(0, nch, piece_ch)):
                    nj = min(piece_ch, nch - j0)
                    q, kk, _ = batch_vals[gbi][jp]
                    outap = (g_sb[:, gbi % nbuf, j0 * 128: (j0 + nj) * 128]
                             .rearrange("p (d f) -> p d f", f=128))
                    if tab32:
                        outap = outap.bitcast(mybir.dt.float32)
                    g.dma_gather(
                        outap,
                        tab[:].rearrange("p c f -> (p c) f"),
                        idx_sb[:, off + j0 * 8: off + j0 * 8 + nj * 8],
                        128 * nj, 128 * nj, tew,
                        queue_num=q, single_packet=spkt,
                    ).then_inc(GS[q][kk], 16)

        @block.sync
        def _(s: bass.BassEngine):
            def write_out(r):
                # result write for rep r (deferred one rep so the sync
                # engine can issue rep r+1's GEMM DMAs before stalling on
                # rep r's DVE completion)
                s.wait_ge(VCH, VTOT * (r + 1))
                s.dma_start(
                    out[:].rearrange("(t p) f -> p t f", p=128),
                    res_sb[:, r % 2],
                ).then_inc(BWOUT, 16)

            s.wait_ge(LIB, 16)
            s.dma_start(w_sb[:], w[:]).then_inc(PRM, 16)
            s.dma_start(idx_sb[:], idx[:]).then_inc(PRM, 16)
            s.dma_start(isdb_sb[:], isdb[:]).then_inc(PRM, 16)
            if xtres and mode not in ("gather", "gatheronly"):
                s.dma_start(xt_sb[:], xt[:]).then_inc(XTL[0], 16)
            for r in range(reps):
                if mode not in ("gather", "gatheronly"):
                    tab = tabs[r % ntab]
                    for ci in range(NCH):
                        gci = r * NCH + ci
                        c0, nt = chunks1[ci]
                        if not xtres:
                            if gci >= 2:
                                s.wait_ge(MMC, gci - 1)  # xt buf parity free
                            s.dma_start(
                                xt_sb[:, gci % 2, : nt * 128],
                                xt[:, c0 * 128: (c0 + nt) * 128],
                            ).then_inc(XTL[gci % 2], 16)
                        if ci == 0 and r >= 2 and mode == "full":
                            # table parity reused: rep r-2's gathers must
                            # be done before overwriting
                            for (q, kk), val in rep_end_vals[r - 2].items():
                                s.wait_ge(GS[q][kk], val)
                        if ci >= 1:
                            pg = gci - 1
                            pc0, pnt = chunks1[ci - 1]
                            s.wait_ge(YCP, pg + 1)
                            tap = tab[:, pc0 // 2: (pc0 + pnt) // 2, :]
                            if tab32:
                                tap = tap.bitcast(mybir.dt.bfloat16)
                            s.dma_start(
                                tap,
                                y_sb[:, pg % 2, : pnt * FOUT]
                                .rearrange("p (c f) -> p c f", f=128),
                            ).then_inc(YWR, 16)
                    gci = r * NCH + NCH - 1
                    lc0, lnt = chunks1[NCH - 1]
                    s.wait_ge(YCP, gci + 1)
                    tap = tab[:, lc0 // 2: (lc0 + lnt) // 2, :]
                    if tab32:
                        tap = tap.bitcast(mybir.dt.bfloat16)
                    s.dma_start(
                        tap,
                        y_sb[:, gci % 2, : lnt * FOUT]
                        .rearrange("p (c f) -> p c f", f=128),
                    ).then_inc(YWR, 16)
                if mode in ("gemm", "gatheronly"):
                    continue
                if r >= 1:
                    write_out(r - 1)
            if mode not in ("gemm", "gatheronly"):
                write_out(reps - 1)

        @block.tensor
        def _(t: bass.BassTensorEngine):
            if mode in ("gather", "gatheronly"):
                return
            t.wait_ge(PRM, 16 * 3)
            if xtres:
                t.wait_ge(XTL[0], 16)
            for gci, (r, ci, c0, nt) in enumerate(GCH):
                if not xtres:
                    t.wait_ge(XTL[gci % 2], 16 * (gci // 2 + 1))
                if gci >= 2:
                    t.wait_ge(YCP, gci - 1)  # psum bank parity free
                ins = None
                for ti in range(nt):
                    lhs = (xt_sb[:, (c0 + ti) * 128: (c0 + ti + 1) * 128]
                           if xtres else
                           xt_sb[:, gci % 2, ti * 128: (ti + 1) * 128])
                    ins = t.matmul(
                        out=ps[:, (gci % 2) * CH_T * FOUT + ti * FOUT:
                               (gci % 2) * CH_T * FOUT + (ti + 1) * FOUT],
                        lhsT=lhs,
                        rhs=w_sb[:],
                        start=True, stop=True,
                    )
                ins.then_inc(MMC, 1)

        @block.scalar
        def _(a: bass.BassScalarEngine):
            if mode in ("gather", "gatheronly"):
                return
            for gci, (r, ci, c0, nt) in enumerate(GCH):
                a.wait_ge(MMC, gci + 1)
                if gci >= 2:
                    a.wait_ge(YWR, 16 * (gci - 1))  # y_sb parity drained
                base = (gci % 2) * CH_T * FOUT
                a.activation(
                    out=y_sb[:, gci % 2, : nt * FOUT],
                    in_=ps[:, base: base + nt * FOUT],
                    func=mybir.ActivationFunctionType.Copy,
                ).then_inc(YCP, 1)

        @block.vector
        def _(v: bass.BassVectorEngine):
            if mode in ("gemm", "gatheronly"):
                return
            vc = [0]

            def chained(fn):
                if vc[0]:
                    v.wait_ge(VCH, vc[0])
                ins = fn()
                ins.then_inc(VCH, 1)
                vc[0] += 1
                return ins

            v.wait_ge(PRM, 16 * 3)
            for gbi, (r, bi, t0, gt, db, off) in enumerate(GBT):
                for (q, kk, val) in batch_vals[gbi]:
                    v.wait_ge(GS[q][kk], val)
                if bi == 0 and r >= 2:
                    v.wait_ge(BWOUT, 16 * (r - 1))  # res parity drained
                buf = g_sb[:, gbi % nbuf, :]
                allslots = buf[:, : db * gt * 128].rearrange(
                    "p (g d f) -> p g d f", g=gt, d=db)
                slots = allslots[:, :, : (db // 2) * 2, :].rearrange(
                    "p g (d two) f -> p g d (two f)", two=2).rearrange(
                    "p g d (four f) -> p g d four f", four=4)
                # slots dims: [128, g, db//2, 4, 64]: (2j,lo) (2j,hi)
                # (2j+1,lo) (2j+1,hi)
                fold = buf[:, FOLD0: FOLD0 + (db // 2) * gt * FOUT].rearrange(
                    "p (g d f) -> p g d f", g=gt, f=FOUT)
                # L0: even slot low half + odd slot high half
                chained(lambda: v.tensor_tensor(
                    out=fold,
                    in0=slots[:, :, :, 0, :],
                    in1=slots[:, :, :, 3, :],
                    op=mybir.AluOpType.add,
                ))
                if db % 2:
                    # odd tail slot (even depth index db-1 -> low half)
                    chained(lambda: v.tensor_tensor(
                        out=fold[:, :, 0, :],
                        in0=fold[:, :, 0, :],
                        in1=allslots[:, :, db - 1, 0:FOUT],
                        op=mybir.AluOpType.add,
                    ))
                cur = db // 2
                while cur > 1:
                    half = cur // 2
                    hi = cur - half
                    chained(lambda half=half, hi=hi: v.tensor_tensor(
                        out=fold[:, :, 0:half, :],
                        in0=fold[:, :, 0:half, :],
                        in1=fold[:, :, hi: hi + half, :],
                        op=mybir.AluOpType.add,
                    ))
                    cur = hi
                chained(lambda: v.tensor_tensor(
                    out=res_sb[:, r % 2, t0: t0 + gt, :],
                    in0=fold[:, :, 0, :],
                    in1=isdb_sb[:, t0: t0 + gt, None].to_broadcast(
                        [128, gt, FOUT]),
                    op=mybir.AluOpType.mult,
                ))
                assert vc[0] == vch_after[bi] + r * VTOT, (gbi, vc[0])

    nc.compile()
    return nc


# --------------------------------- kernel -----------------------------------
def prepare(edges, features, W2, b2):
    """Build (nc, in_maps, assemble) for the given full inputs."""
    edges = np.asarray(edges)
    X = np.asarray(features, dtype=np.float32)
    W2 = np.asarray(W2, dtype=np.float32)
    b2 = np.asarray(b2, dtype=np.float32)
    src = edges[0].astype(np.int64)
    dst = edges[1].astype(np.int64)

    isd, color, pos_in_color, batches, inmaps, outmaps = _build_schedule(
        src, dst)

    key = ("v2", tuple(batches))
    if key not in _cache:
        _cache[key] = _build_program(batches)
    nc = _cache[key]

    # XT columns: tile 2c -> color0 nodes [c*128:(c+1)*128], tile 2c+1 ->
    # color1 same range (pads zero)
    Xs = (X * isd[:, None]).astype(np.float32)
    XT = np.zeros((FIN, NCOL), dtype=np.float32)
    pos = pos_in_color
    colc = (pos // 128) * 2 + color  # tile index of node
    coli = colc * 128 + (pos % 128)
    XT[:, coli] = Xs.T
    XTb = XT.astype(ml_dtypes.bfloat16)
    Wb = W2.astype(ml_dtypes.bfloat16)

    in_maps = []
    for k in range(NCORES):
        m = dict(inmaps[k])
        m["xt"] = XTb
        m["w"] = Wb
        in_maps.append(m)

    def assemble(results):
        z = np.empty((N, FOUT), dtype=np.float32)
        for k in range(NCORES):
            om = outmaps[k]
            valid = om >= 0
            z[om[valid]] = results[k]["out"][valid]
        return z + b2[None, :]

    return nc, in_maps, assemble


def kernel(edges, features, W1, b1, W2, b2):
    nc, in_maps, assemble = prepare(edges, features, W2, b2)
    res = run_bass_kernel_spmd(nc, in_maps, list(range(NCORES)))
    return assemble(res.results)
